# revision 4
# baseline (speedup 1.0000x reference)
"""MultiHeadAttention Bass/Tile kernel for Trainium2, 8 NeuronCores. V2.

Sharding: (batch, query-half) -> 8 cores, zero collectives.
  core c: batch b = c//2, query rows qh = c%2 (1024 rows each).

All inputs are converted to fp16 and pre-transposed on HOST, so every matmul
operand loads with its contraction dim on partitions and DMA traffic is half
of f32. The mask is sent as (1-mask) in fp16 ("keep" multiplier).

Per-core dataflow:
  P0: stage QA/KTb/MB/weights; Q-proj + K-proj for head-pair 0.
  P1: per head-pair eo (8): per head, per k-strip: S^T[k,q] = K_h^T.T @ Q_h^T
      -> exp(S^T/8) fp16 (ACT) -> * (1-mask^T) (DVE) = pb
      -> PV in [q,d] layout: pv[q, qc, d|den] += pb_chunk.T @ V[strip, h]
         (65-wide moving, full 128 output partitions, ones col -> denominator)
      V-proj is software-pipelined INTO (eo0,h0)'s strip loop; Q/K-proj for
      eo+1 interleaved into the strip stream; PV lags scores by one strip so
      the PE never waits on exp.  Head epilogue: r = 1/den (DVE), normalize
      pv -> OAn fp16 via tensor_scalar with per-partition scalar (fused evac).
  P2: PE-transpose OAn -> OA^T (fp16, via identity), y = OA^T.T @ WO per
      q-tile, DMA y straight from PSUM.
"""

import os
import sys

for _p in ("/opt/trn_rl_repo", "/root/.axon_site/_ro/trn_rl_repo"):
    if os.path.isdir(_p) and _p not in sys.path:
        sys.path.insert(0, _p)

from contextlib import ExitStack

import numpy as np

import concourse.tile as tile
from concourse import bacc, masks, mybir
from concourse.bass_utils import run_bass_kernel_spmd

B, S, D = 4, 2048, 1024
H, HD = 16, 64
Q = S // 2  # per-core query rows
NCORES = 8
NKS = S // 128  # 16 k-strips
NQC = Q // 128  # 8 q-chunks

F32 = mybir.dt.float32
F16 = mybir.dt.float16

_NC_CACHE = {}


def _build_kernel(tc, t_in, t_out, phases="all"):
    nc = tc.nc
    qT, kT, vT, mbT = t_in["qT"], t_in["kT"], t_in["vT"], t_in["mbT"]
    wqT, wkT, wvT, woT = t_in["wqT"], t_in["wkT"], t_in["wvT"], t_in["woT"]
    y = t_out["y"]

    qT3 = qT[:, :].rearrange("(po pi) q -> pi po q", pi=128)  # [128, 8, Q]
    kT3 = kT[:, :].rearrange("(po pi) s -> pi po s", pi=128)
    vT3 = vT[:, :].rearrange("(po pi) s -> pi po s", pi=128)
    mb3 = mbT[:, :].rearrange("(ko ki) q -> ki ko q", ki=128)  # [128, 16, Q]
    wq3 = wqT[:, :].rearrange("(po pi) e -> pi po e", pi=128)
    wk3 = wkT[:, :].rearrange("(po pi) e -> pi po e", pi=128)
    wv3 = wvT[:, :].rearrange("(po pi) e -> pi po e", pi=128)
    wo3 = woT[:, :].rearrange("(po pi) e -> pi po e", pi=128)

    with ExitStack() as ctx:
        # ---- persistent SBUF ----
        p1 = ctx.enter_context(tc.tile_pool(name="persist1", bufs=1))
        QT = p1.tile([128, 8, Q], F16)  # Q^T[e%128, e//128, q]
        V = p1.tile([128, NKS, H, 66], F16)  # [s%128, strip, h, d | one | pad]
        MB = p1.tile([128, NKS, Q], F16)  # (1-mask)^T strips
        OAn = p1.tile([128, NQC, D], F16)  # normalized attn out [q%128, qc, (h d)]
        ident = p1.tile([128, 128], F16)  # for PE transposes

        pctx = ctx.enter_context(ExitStack())
        p2p = pctx.enter_context(tc.tile_pool(name="persist2", bufs=1))
        QA = p2p.tile([128, 8, Q], F16)  # q^T staged
        KTb = p2p.tile([128, 8, S], F16)  # k^T staged

        nc.vector.memset(V[:, :, :, 64:65], 1.0)
        masks.make_identity(nc, ident[:, :])

        kte_pool = pctx.enter_context(tc.tile_pool(name="kte", bufs=3))
        # per-eo slices of wq/wk (only 128 e-cols needed per head-pair)
        wqp = pctx.enter_context(tc.tile_pool(name="wqp", bufs=2))
        wkp = pctx.enter_context(tc.tile_pool(name="wkp", bufs=2))

        WQe, WKe = {}, {}

        def load_w(eo):
            WQe[eo] = wqp.tile([128, 8, 128], F16, tag="wq", name=f"wq{eo}")
            nc.sync.dma_start(
                out=WQe[eo], in_=wq3[:, :, eo * 128 : (eo + 1) * 128]
            )
            WKe[eo] = wkp.tile([128, 8, 128], F16, tag="wk", name=f"wk{eo}")
            nc.sync.dma_start(
                out=WKe[eo], in_=wk3[:, :, eo * 128 : (eo + 1) * 128]
            )

        with (
            tc.tile_pool(name="wv", bufs=1) as wvp,
            tc.tile_pool(name="va", bufs=2) as vap,
            tc.tile_pool(name="pbuf", bufs=8) as pbuf,
            tc.tile_pool(name="rbuf", bufs=2) as rbuf,
            tc.tile_pool(name="psS", bufs=2, space="PSUM") as psS,  # 4 banks
            tc.tile_pool(name="psPV", bufs=1, space="PSUM") as psPV,  # 2
            tc.tile_pool(name="pj", bufs=1, space="PSUM") as pj,  # 2 banks
        ):
            WV = wvp.tile([128, 8, D], F16)

            KTe = {}  # eo -> [128, S] f16 K^T chunk (ring of 2)

            def q_proj_grp(eo):
                # QT[e-chunk eo, :] = sum_d wq[d, e] qa[d, q]
                ps = pj.tile([128, Q], F32, tag="pj", name="qps")
                for qn in range(2):
                    for dc in range(8):
                        nc.tensor.matmul(
                            ps[:, qn * 512 : (qn + 1) * 512],
                            WQe[eo][:, dc, :],
                            QA[:, dc, qn * 512 : (qn + 1) * 512],
                            start=(dc == 0),
                            stop=(dc == 7),
                        )
                nc.vector.tensor_copy(out=QT[:, eo, :], in_=ps)

            def k_proj_grp(eo, snp):
                ps = pj.tile([128, Q], F32, tag="pj", name="kps")
                for sn2 in range(2):
                    sn = snp * 2 + sn2
                    for dc in range(8):
                        nc.tensor.matmul(
                            ps[:, sn2 * 512 : (sn2 + 1) * 512],
                            WKe[eo][:, dc, :],
                            KTb[:, dc, sn * 512 : (sn + 1) * 512],
                            start=(dc == 0),
                            stop=(dc == 7),
                        )
                nc.vector.tensor_copy(
                    out=KTe[eo][:, snp * 1024 : (snp + 1) * 1024], in_=ps
                )

            VA = {}

            def load_va(sn):
                VA[sn] = vap.tile([128, 8, 256], F16, tag="va", name=f"va{sn}")
                nc.sync.dma_start(
                    out=VA[sn], in_=vT3[:, :, sn * 256 : (sn + 1) * 256]
                )

            def v_proj_grp(st):
                # V[strip st, :, :] = sum_d v^T[d, s].T @ wv[d, e]
                ps = pj.tile([128, D], F32, tag="pj", name="vps")
                va, stl = VA[st // 2], st % 2
                for en in range(2):
                    for dc in range(8):
                        nc.tensor.matmul(
                            ps[:, en * 512 : (en + 1) * 512],
                            va[:, dc, stl * 128 : (stl + 1) * 128],
                            WV[:, dc, en * 512 : (en + 1) * 512],
                            start=(dc == 0),
                            stop=(dc == 7),
                        )
                nc.scalar.copy(
                    out=V[:, st, :, 0:64],
                    in_=ps[:, :].rearrange("p (h d) -> p h d", h=16),
                )

            # ---- P0: stage + projections for head-pair 0 ----
            # DMAs ordered by first use so the PE starts ASAP: wk0/wq0 and
            # the first KTb chunk unblock k_proj(0,0) within a few us.
            nc.sync.dma_start(
                out=KTb[:, :, 0:512], in_=kT3[:, :, 0:512]
            )
            load_w(0)
            nc.sync.dma_start(out=QA, in_=qT3)
            for sn in range(1, 4):
                nc.sync.dma_start(
                    out=KTb[:, :, sn * 512 : (sn + 1) * 512],
                    in_=kT3[:, :, sn * 512 : (sn + 1) * 512],
                )
            load_w(1)
            nc.sync.dma_start(out=WV, in_=wv3)
            load_va(0)
            load_va(1)
            nc.sync.dma_start(out=MB[:, 0:4, :], in_=mb3[:, 0:4, :])
            nc.sync.dma_start(out=MB[:, 4:16, :], in_=mb3[:, 4:16, :])
            KTe[0] = kte_pool.tile([128, S], F16, tag="kte", name="kte0")
            for snp in range(2):
                k_proj_grp(0, snp)
            q_proj_grp(0)

            if phases == "p0":
                return

            # ---- P1 ----
            # filler: PE work + prefetch DMAs to interleave into the strip
            # stream.  DMA items lead their consumers by >= one group.
            def filler_gen():
                # eo0 h0: V-projection, strip-by-strip (strip st done well
                # before PV(h0, st) consumes it at slot st+2).
                for sn in range(8):
                    if sn + 2 < 8:
                        yield ("vd", sn + 2)
                    for stl in range(2):
                        yield ("v", sn * 2 + stl)
                # eo 1..7 prep; weight slices for eo+1 prefetched while eo's
                # groups are computed.
                for eo in range(1, 8):
                    if eo + 1 < 8:
                        yield ("w", eo + 1)
                    KTe[eo] = kte_pool.tile(
                        [128, S], F16, tag="kte", name=f"kte{eo}"
                    )
                    for snp in range(2):
                        yield ("k", eo, snp)
                    yield ("q", eo)

            fill = filler_gen()

            def t_head(hi):
                # estimated start time (ns) of head hi in the floor schedule
                return 13000 + (58000 if hi >= 1 else 0) + max(0, hi - 1) * 18000

            def do_fill(n):
                for _ in range(n):
                    item = next(fill, None)
                    if item is None:
                        return
                    kind = item[0]
                    if kind == "v":
                        v_proj_grp(item[1])
                    elif kind == "vd":
                        load_va(item[1])
                    elif kind == "w":
                        eo2 = item[1]
                        with tc.tile_wait_until(t_head(max(0, 2 * eo2 - 4)) / 1e6):
                            load_w(eo2)
                    elif kind == "k":
                        eo2, snp = item[1], item[2]
                        base = t_head(2 * eo2 - 2 if eo2 > 1 else 1)
                        with tc.tile_wait_until((base + snp * 5000) / 1e6):
                            k_proj_grp(eo2, snp)
                    else:
                        eo2 = item[1]
                        base = t_head(2 * eo2 - 2 if eo2 > 1 else 1)
                        with tc.tile_wait_until((base + 11000) / 1e6):
                            q_proj_grp(eo2)

            def transpose_grp(j, qt):
                # in-place: OAn chunk (j, qt) -> its own transpose via PSUM
                tp = pj.tile([128, 128], F16, tag="pj", name="tp")
                nc.tensor.transpose(
                    tp, OAn[:, qt, j * 128 : (j + 1) * 128], ident[:, :]
                )
                nc.vector.tensor_copy(
                    out=OAn[:, qt, j * 128 : (j + 1) * 128], in_=tp
                )

            def do_pv(pv, h, ks, pb):
                # pv is 2 psum banks; 4 qc-slabs share a 2KB zero region ->
                # start only on the first matmul touching the bank (zeroes
                # the whole region), stop on the last.
                for qc in range(NQC):
                    nc.tensor.matmul(
                        pv[:, qc, 0:65],
                        pb[:, qc * 128 : (qc + 1) * 128],
                        V[:, ks, h, 0:65],
                        start=(ks == 0 and qc % 4 == 0),
                        stop=(ks == NKS - 1 and qc % 4 == 3),
                    )

            def epilogue(pv, h):
                # normalize pv -> OAn, fused with the PSUM evacuation.  On
                # ACT (Copy with per-partition scale) so the DVE queue (busy
                # with masks) doesn't delay the next head's PV.
                rsb = rbuf.tile([128, NQC], F32, tag="r")
                nc.vector.reciprocal(
                    out=rsb,
                    in_=pv[:, :, 64:65].rearrange("p a b -> p (a b)"),
                )
                for qc in range(NQC):
                    nc.vector.tensor_scalar(
                        out=OAn[:, qc, h * 64 : (h + 1) * 64],
                        in0=pv[:, qc, 0:64],
                        scalar1=rsb[:, qc : qc + 1],
                        scalar2=None,
                        op0=mybir.AluOpType.mult,
                    )

            # Flat software pipeline over all (head, strip) slots.  The PV
            # queue (lag 2) spans head boundaries so the next head's scores
            # never sit behind the previous head's tail PVs in the in-order
            # PE queue.  Epilogue runs when a head's last PV retires; the pv
            # psum buffer is re-allocated when the next head's first PV pops.
            state = {"pv": None, "epi": None}

            def pop_pv(pend):
                h2, ks2, pb2 = pend.pop(0)
                if ks2 == 0:
                    if state["epi"] is not None:
                        epilogue(*state["epi"])
                        state["epi"] = None
                    state["pv"] = psPV.tile(
                        [128, NQC, 128], F32, tag="pv", name="pv"
                    )
                do_pv(state["pv"], h2, ks2, pb2)
                if ks2 == NKS - 1:
                    state["epi"] = (state["pv"], h2)

            pend = []
            for slot, (eo, hl, ks) in enumerate(
                (e, l, k) for e in range(8) for l in range(2) for k in range(NKS)
            ):
                h = 2 * eo + hl
                hp = 64 * hl
                # PE-ready work first: while ACT finishes exp(slot-2)
                # (freeing the sps buffer this slot's scores need), the
                # in-order PE queue chews through filler + lagged PV.
                if eo == 0 and hl == 0:
                    do_fill(2)
                elif (ks % 2) == 1:
                    do_fill(1)
                if len(pend) >= 4:
                    pop_pv(pend)
                # in-place transposes of head-pair eo-1 (its OAn chunk is
                # final: head 2eo-1's epilogue was emitted at slot 2 above)
                if hl == 0 and eo >= 1 and 4 <= ks < 4 + NQC:
                    transpose_grp(eo - 1, ks - 4)
                sps = psS.tile([128, Q], F32, tag="ps", name="sps")
                lhsT = KTe[eo][hp : hp + 64, ks * 128 : (ks + 1) * 128]
                for qn in range(2):
                    nc.tensor.matmul(
                        sps[:, qn * 512 : (qn + 1) * 512],
                        lhsT,
                        QT[hp : hp + 64, eo, qn * 512 : (qn + 1) * 512],
                        start=True,
                        stop=True,
                    )
                pb = pbuf.tile([128, Q], F16, tag="pb")
                nc.scalar.activation(
                    out=pb,
                    in_=sps,
                    func=mybir.ActivationFunctionType.Exp,
                    scale=0.125,
                )
                nc.vector.tensor_tensor(
                    out=pb, in0=pb, in1=MB[:, ks, :],
                    op=mybir.AluOpType.mult,
                )
                pend.append((h, ks, pb))
            while pend:
                pop_pv(pend)
            do_fill(999)  # drain any leftover filler (incl. transposes)
            epilogue(*state["epi"])
            # last head-pair's transposes (OAn chunk 7 final after epilogue)
            for qt in range(NQC):
                transpose_grp(7, qt)

        if phases == "p0p1":
            return
        pctx.close()  # QA/KTb/WQ/WK/KTe dead -> release SBUF

        # ---- P2: y = OA^T.T @ WO (OAn holds OA^T in-place) ----
        with (
            tc.tile_pool(name="p2", bufs=1) as p2,
            tc.tile_pool(name="ybuf", bufs=2) as ybuf,
            tc.tile_pool(name="psY", bufs=2, space="PSUM") as psY,
        ):
            WO = p2.tile([128, 8, D], F16)
            nc.sync.dma_start(out=WO, in_=wo3)

            for qt in range(NQC):
                yps = psY.tile([128, D], F32, tag="y")
                for j in range(8):
                    for en in range(2):
                        nc.tensor.matmul(
                            yps[:, en * 512 : (en + 1) * 512],
                            OAn[:, qt, j * 128 : (j + 1) * 128],
                            WO[:, j, en * 512 : (en + 1) * 512],
                            start=(j == 0),
                            stop=(j == 7),
                        )
                yb = ybuf.tile([128, D], F32, tag="yb")
                nc.vector.tensor_copy(out=yb, in_=yps)
                nc.sync.dma_start(out=y[qt * 128 : (qt + 1) * 128, :], in_=yb)


def _get_nc(phases="all"):
    if phases in _NC_CACHE:
        return _NC_CACHE[phases]
    nc = bacc.Bacc("TRN2", target_bir_lowering=False)
    t_in = {
        "qT": nc.dram_tensor("qT", [D, Q], F16, kind="ExternalInput"),
        "kT": nc.dram_tensor("kT", [D, S], F16, kind="ExternalInput"),
        "vT": nc.dram_tensor("vT", [D, S], F16, kind="ExternalInput"),
        "mbT": nc.dram_tensor("mbT", [S, Q], F16, kind="ExternalInput"),
        "wqT": nc.dram_tensor("wqT", [D, D], F16, kind="ExternalInput"),
        "wkT": nc.dram_tensor("wkT", [D, D], F16, kind="ExternalInput"),
        "wvT": nc.dram_tensor("wvT", [D, D], F16, kind="ExternalInput"),
        "woT": nc.dram_tensor("woT", [D, D], F16, kind="ExternalInput"),
    }
    t_out = {"y": nc.dram_tensor("y", [Q, D], F32, kind="ExternalOutput")}
    with tile.TileContext(nc) as tc:
        _build_kernel(tc, t_in, t_out, phases=phases)
    nc.compile()
    _NC_CACHE[phases] = nc
    return nc


def _in_maps(inputs):
    q = np.asarray(inputs["query"], np.float32)
    k = np.asarray(inputs["key"], np.float32)
    v = np.asarray(inputs["value"], np.float32)
    mask = np.asarray(inputs["mask"], np.int32)
    f16 = np.float16
    wqT = np.ascontiguousarray(np.asarray(inputs["wq"], np.float32).T).astype(f16)
    wkT = np.ascontiguousarray(np.asarray(inputs["wk"], np.float32).T).astype(f16)
    wvT = np.ascontiguousarray(np.asarray(inputs["wv"], np.float32).T).astype(f16)
    woT = np.ascontiguousarray(np.asarray(inputs["w_out"], np.float32).T).astype(f16)
    maps = []
    for c in range(NCORES):
        b, qh = c // 2, c % 2
        sl = slice(qh * Q, (qh + 1) * Q)
        maps.append(
            {
                "qT": np.ascontiguousarray(q[b].T[:, sl]).astype(f16),
                "kT": np.ascontiguousarray(k[b].T).astype(f16),
                "vT": np.ascontiguousarray(v[b].T).astype(f16),
                "mbT": np.ascontiguousarray(
                    (1 - mask[b].T[:, sl]).astype(f16)
                ),
                "wqT": wqT,
                "wkT": wkT,
                "wvT": wvT,
                "woT": woT,
            }
        )
    return maps


def _gather(res):
    outs = [res.results[c]["y"] for c in range(NCORES)]
    return np.stack(
        [np.concatenate([outs[2 * b], outs[2 * b + 1]], axis=0) for b in range(B)]
    )


def kernel(**inputs) -> np.ndarray:
    nc = _get_nc()
    res = run_bass_kernel_spmd(nc, _in_maps(inputs), core_ids=list(range(NCORES)))
    return _gather(res)


def kernel_traced(**inputs):
    """Like kernel() but with NTFF tracing; returns (output, BassKernelResults)."""
    nc = _get_nc()
    res = run_bass_kernel_spmd(
        nc, _in_maps(inputs), core_ids=list(range(NCORES)), trace=True
    )
    return _gather(res), res


# revision 5
# speedup vs baseline: 1.0211x; 1.0211x over previous
"""MultiHeadAttention Bass/Tile kernel for Trainium2, 8 NeuronCores. V2.

Sharding: (batch, query-half) -> 8 cores, zero collectives.
  core c: batch b = c//2, query rows qh = c%2 (1024 rows each).

All inputs are converted to fp16 and pre-transposed on HOST, so every matmul
operand loads with its contraction dim on partitions and DMA traffic is half
of f32. The mask is sent as (1-mask) in fp16 ("keep" multiplier).

Per-core dataflow:
  P0: stage QA/KTb/MB/weights (DMAs ordered by first use); Q/K-proj for
      head-pair 0.
  P1: one flat software pipeline over all 256 (head, k-strip) slots:
      S^T[k,q] = K_h^T.T @ Q_h^T -> exp(S^T/8) fp16 (ACT)
      -> *= (1-mask^T) in place (DVE) = pb
      -> PV in [q,d] layout: pv[q, qc, d|den] += pb_chunk.T @ V[strip, h]
         (65-wide moving, full 128 output partitions, ones col -> denom;
         one PSUM-zero-region start/stop per bank).
      The PV queue lags scores by 4 slots and spans head boundaries.  V-proj
      is interleaved into (eo0,h0)'s strips; K/Q-proj for eo+1 and the
      in-place PE transposes of finished OAn chunks ride in later slots.
      Head epilogue (when its last PV pops): r = 1/den (DVE reciprocal),
      normalize pv -> OAn fp16 via tensor_scalar per-partition scale.
  P2: y[qt] = OA^T.T @ WO only (transposes already done in P1).
"""

import os
import sys

for _p in ("/opt/trn_rl_repo", "/root/.axon_site/_ro/trn_rl_repo"):
    if os.path.isdir(_p) and _p not in sys.path:
        sys.path.insert(0, _p)

from contextlib import ExitStack

import numpy as np

import concourse.tile as tile
from concourse import bacc, masks, mybir
from concourse.bass_utils import run_bass_kernel_spmd

B, S, D = 4, 2048, 1024
H, HD = 16, 64
Q = S // 2  # per-core query rows
NCORES = 8
NKS = S // 128  # 16 k-strips
NQC = Q // 128  # 8 q-chunks

F32 = mybir.dt.float32
F16 = mybir.dt.float16

_NC_CACHE = {}


def _build_kernel(tc, t_in, t_out, phases="all"):
    nc = tc.nc
    qT, kT, vT, mbT = t_in["qT"], t_in["kT"], t_in["vT"], t_in["mbT"]
    wqT, wkT, wvT, woT = t_in["wqT"], t_in["wkT"], t_in["wvT"], t_in["woT"]
    y = t_out["y"]

    qT3 = qT[:, :].rearrange("(po pi) q -> pi po q", pi=128)  # [128, 8, Q]
    kT3 = kT[:, :].rearrange("(po pi) s -> pi po s", pi=128)
    vT3 = vT[:, :].rearrange("(po pi) s -> pi po s", pi=128)
    mb3 = mbT[:, :].rearrange("(ko ki) q -> ki ko q", ki=128)  # [128, 16, Q]
    wq3 = wqT[:, :].rearrange("(po pi) e -> pi po e", pi=128)
    wk3 = wkT[:, :].rearrange("(po pi) e -> pi po e", pi=128)
    wv3 = wvT[:, :].rearrange("(po pi) e -> pi po e", pi=128)
    wo3 = woT[:, :].rearrange("(po pi) e -> pi po e", pi=128)

    with ExitStack() as ctx:
        # ---- persistent SBUF ----
        p1 = ctx.enter_context(tc.tile_pool(name="persist1", bufs=1))
        QT = p1.tile([128, 8, Q], F16)  # Q^T[e%128, e//128, q]
        V = p1.tile([128, NKS, H, 66], F16)  # [s%128, strip, h, d | one | pad]
        MB = p1.tile([128, NKS, Q], F16)  # (1-mask)^T strips
        OAn = p1.tile([128, NQC, D], F16)  # normalized attn out [q%128, qc, (h d)]
        ident = p1.tile([128, 128], F16)  # for PE transposes

        pctx = ctx.enter_context(ExitStack())
        p2p = pctx.enter_context(tc.tile_pool(name="persist2", bufs=1))
        QA = p2p.tile([128, 8, Q], F16)  # q^T staged
        KTb = p2p.tile([128, 8, S], F16)  # k^T staged

        nc.vector.memset(V[:, :, :, 64:65], 1.0)
        masks.make_identity(nc, ident[:, :])

        kte_pool = pctx.enter_context(tc.tile_pool(name="kte", bufs=3))
        # per-eo slices of wq/wk (only 128 e-cols needed per head-pair)
        wqp = pctx.enter_context(tc.tile_pool(name="wqp", bufs=2))
        wkp = pctx.enter_context(tc.tile_pool(name="wkp", bufs=2))

        WQe, WKe = {}, {}

        def load_w(eo):
            WQe[eo] = wqp.tile([128, 8, 128], F16, tag="wq", name=f"wq{eo}")
            nc.sync.dma_start(
                out=WQe[eo], in_=wq3[:, :, eo * 128 : (eo + 1) * 128]
            )
            WKe[eo] = wkp.tile([128, 8, 128], F16, tag="wk", name=f"wk{eo}")
            nc.sync.dma_start(
                out=WKe[eo], in_=wk3[:, :, eo * 128 : (eo + 1) * 128]
            )

        with (
            tc.tile_pool(name="wv", bufs=1) as wvp,
            tc.tile_pool(name="va", bufs=2) as vap,
            tc.tile_pool(name="pbuf", bufs=8) as pbuf,
            tc.tile_pool(name="rbuf", bufs=2) as rbuf,
            tc.tile_pool(name="psS", bufs=2, space="PSUM") as psS,  # 4 banks
            tc.tile_pool(name="psPV", bufs=1, space="PSUM") as psPV,  # 2
            tc.tile_pool(name="pj", bufs=1, space="PSUM") as pj,  # 2 banks
        ):
            WV = wvp.tile([128, 8, D], F16)

            KTe = {}  # eo -> [128, S] f16 K^T chunk (ring of 2)

            def q_proj_grp(eo):
                # QT[e-chunk eo, :] = sum_d wq[d, e] qa[d, q]
                ps = pj.tile([128, Q], F32, tag="pj", name="qps")
                for qn in range(2):
                    for dc in range(8):
                        nc.tensor.matmul(
                            ps[:, qn * 512 : (qn + 1) * 512],
                            WQe[eo][:, dc, :],
                            QA[:, dc, qn * 512 : (qn + 1) * 512],
                            start=(dc == 0),
                            stop=(dc == 7),
                        )
                nc.vector.tensor_copy(out=QT[:, eo, :], in_=ps)

            def k_proj_grp(eo, snp):
                ps = pj.tile([128, Q], F32, tag="pj", name="kps")
                for sn2 in range(2):
                    sn = snp * 2 + sn2
                    for dc in range(8):
                        nc.tensor.matmul(
                            ps[:, sn2 * 512 : (sn2 + 1) * 512],
                            WKe[eo][:, dc, :],
                            KTb[:, dc, sn * 512 : (sn + 1) * 512],
                            start=(dc == 0),
                            stop=(dc == 7),
                        )
                nc.vector.tensor_copy(
                    out=KTe[eo][:, snp * 1024 : (snp + 1) * 1024], in_=ps
                )

            VA = {}

            def load_va(sn):
                VA[sn] = vap.tile([128, 8, 256], F16, tag="va", name=f"va{sn}")
                nc.sync.dma_start(
                    out=VA[sn], in_=vT3[:, :, sn * 256 : (sn + 1) * 256]
                )

            def v_proj_grp(st):
                # V[strip st, :, :] = sum_d v^T[d, s].T @ wv[d, e]
                ps = pj.tile([128, D], F32, tag="pj", name="vps")
                va, stl = VA[st // 2], st % 2
                for en in range(2):
                    for dc in range(8):
                        nc.tensor.matmul(
                            ps[:, en * 512 : (en + 1) * 512],
                            va[:, dc, stl * 128 : (stl + 1) * 128],
                            WV[:, dc, en * 512 : (en + 1) * 512],
                            start=(dc == 0),
                            stop=(dc == 7),
                        )
                nc.scalar.copy(
                    out=V[:, st, :, 0:64],
                    in_=ps[:, :].rearrange("p (h d) -> p h d", h=16),
                )

            # ---- P0: stage + projections for head-pair 0 ----
            # DMAs ordered by first use so the PE starts ASAP: wk0/wq0 and
            # the first KTb chunk unblock k_proj(0,0) within a few us.
            nc.sync.dma_start(
                out=KTb[:, :, 0:512], in_=kT3[:, :, 0:512]
            )
            load_w(0)
            nc.sync.dma_start(out=QA, in_=qT3)
            for sn in range(1, 4):
                nc.sync.dma_start(
                    out=KTb[:, :, sn * 512 : (sn + 1) * 512],
                    in_=kT3[:, :, sn * 512 : (sn + 1) * 512],
                )
            load_w(1)
            nc.sync.dma_start(out=WV, in_=wv3)
            load_va(0)
            load_va(1)
            nc.sync.dma_start(out=MB[:, 0:4, :], in_=mb3[:, 0:4, :])
            nc.sync.dma_start(out=MB[:, 4:16, :], in_=mb3[:, 4:16, :])
            KTe[0] = kte_pool.tile([128, S], F16, tag="kte", name="kte0")
            for snp in range(2):
                k_proj_grp(0, snp)
            q_proj_grp(0)

            if phases == "p0":
                return

            # ---- P1 ----
            # filler: PE work + prefetch DMAs to interleave into the strip
            # stream.  DMA items lead their consumers by >= one group.
            def filler_gen():
                # eo0 h0: V-projection, strip-by-strip (strip st done well
                # before PV(h0, st) consumes it at slot st+2).
                for sn in range(8):
                    if sn + 2 < 8:
                        yield ("vd", sn + 2)
                    for stl in range(2):
                        yield ("v", sn * 2 + stl)
                # eo 1..7 prep; weight slices for eo+1 prefetched while eo's
                # groups are computed.
                for eo in range(1, 8):
                    if eo + 1 < 8:
                        yield ("w", eo + 1)
                    KTe[eo] = kte_pool.tile(
                        [128, S], F16, tag="kte", name=f"kte{eo}"
                    )
                    for snp in range(2):
                        yield ("k", eo, snp)
                    yield ("q", eo)

            fill = filler_gen()

            def t_head(hi):
                # estimated start time (ns) of head hi in the floor schedule
                return 13000 + (58000 if hi >= 1 else 0) + max(0, hi - 1) * 18000

            def do_fill(n):
                for _ in range(n):
                    item = next(fill, None)
                    if item is None:
                        return
                    kind = item[0]
                    if kind == "v":
                        v_proj_grp(item[1])
                    elif kind == "vd":
                        load_va(item[1])
                    elif kind == "w":
                        eo2 = item[1]
                        with tc.tile_wait_until(t_head(max(0, 2 * eo2 - 4)) / 1e6):
                            load_w(eo2)
                    elif kind == "k":
                        eo2, snp = item[1], item[2]
                        base = t_head(2 * eo2 - 2 if eo2 > 1 else 1)
                        with tc.tile_wait_until((base + snp * 5000) / 1e6):
                            k_proj_grp(eo2, snp)
                    else:
                        eo2 = item[1]
                        base = t_head(2 * eo2 - 2 if eo2 > 1 else 1)
                        with tc.tile_wait_until((base + 11000) / 1e6):
                            q_proj_grp(eo2)

            def transpose_grp(j, qt):
                # in-place: OAn chunk (j, qt) -> its own transpose via PSUM
                tp = pj.tile([128, 128], F16, tag="pj", name="tp")
                nc.tensor.transpose(
                    tp, OAn[:, qt, j * 128 : (j + 1) * 128], ident[:, :]
                )
                nc.vector.tensor_copy(
                    out=OAn[:, qt, j * 128 : (j + 1) * 128], in_=tp
                )

            def do_pv(pv, h, ks, pb):
                # pv is 2 psum banks; 4 qc-slabs share a 2KB zero region ->
                # start only on the first matmul touching the bank (zeroes
                # the whole region), stop on the last.
                for qc in range(NQC):
                    nc.tensor.matmul(
                        pv[:, qc, 0:65],
                        pb[:, qc * 128 : (qc + 1) * 128],
                        V[:, ks, h, 0:65],
                        start=(ks == 0 and qc % 4 == 0),
                        stop=(ks == NKS - 1 and qc % 4 == 3),
                    )

            def epilogue(pv, h):
                # normalize pv -> OAn, fused with the PSUM evacuation.  On
                # ACT (Copy with per-partition scale) so the DVE queue (busy
                # with masks) doesn't delay the next head's PV.
                rsb = rbuf.tile([128, NQC], F32, tag="r")
                nc.vector.reciprocal(
                    out=rsb,
                    in_=pv[:, :, 64:65].rearrange("p a b -> p (a b)"),
                )
                for qc in range(NQC):
                    nc.vector.tensor_scalar(
                        out=OAn[:, qc, h * 64 : (h + 1) * 64],
                        in0=pv[:, qc, 0:64],
                        scalar1=rsb[:, qc : qc + 1],
                        scalar2=None,
                        op0=mybir.AluOpType.mult,
                    )

            # Flat software pipeline over all (head, strip) slots.  The PV
            # queue (lag 2) spans head boundaries so the next head's scores
            # never sit behind the previous head's tail PVs in the in-order
            # PE queue.  Epilogue runs when a head's last PV retires; the pv
            # psum buffer is re-allocated when the next head's first PV pops.
            state = {"pv": None, "epi": None}

            def pop_pv(pend):
                h2, ks2, pb2 = pend.pop(0)
                if ks2 == 0:
                    if state["epi"] is not None:
                        epilogue(*state["epi"])
                        state["epi"] = None
                    state["pv"] = psPV.tile(
                        [128, NQC, 128], F32, tag="pv", name="pv"
                    )
                do_pv(state["pv"], h2, ks2, pb2)
                if ks2 == NKS - 1:
                    state["epi"] = (state["pv"], h2)

            pend = []
            for slot, (eo, hl, ks) in enumerate(
                (e, l, k) for e in range(8) for l in range(2) for k in range(NKS)
            ):
                h = 2 * eo + hl
                hp = 64 * hl
                # PE-ready work first: while ACT finishes exp(slot-2)
                # (freeing the sps buffer this slot's scores need), the
                # in-order PE queue chews through filler + lagged PV.
                if eo == 0 and hl == 0:
                    do_fill(2)
                elif (ks % 2) == 1:
                    do_fill(1)
                if len(pend) >= 4:
                    pop_pv(pend)
                # in-place transposes of head-pair eo-1 (its OAn chunk is
                # final: head 2eo-1's epilogue was emitted at slot 2 above)
                if hl == 0 and eo >= 1 and 4 <= ks < 4 + NQC:
                    transpose_grp(eo - 1, ks - 4)
                sps = psS.tile([128, Q], F32, tag="ps", name="sps")
                lhsT = KTe[eo][hp : hp + 64, ks * 128 : (ks + 1) * 128]
                for qn in range(2):
                    nc.tensor.matmul(
                        sps[:, qn * 512 : (qn + 1) * 512],
                        lhsT,
                        QT[hp : hp + 64, eo, qn * 512 : (qn + 1) * 512],
                        start=True,
                        stop=True,
                    )
                pb = pbuf.tile([128, Q], F16, tag="pb")
                nc.scalar.activation(
                    out=pb,
                    in_=sps,
                    func=mybir.ActivationFunctionType.Exp,
                    scale=0.125,
                )
                nc.vector.tensor_tensor(
                    out=pb, in0=pb, in1=MB[:, ks, :],
                    op=mybir.AluOpType.mult,
                )
                pend.append((h, ks, pb))
            while pend:
                pop_pv(pend)
            do_fill(999)  # drain any leftover filler (incl. transposes)
            epilogue(*state["epi"])
            # last head-pair's transposes (OAn chunk 7 final after epilogue)
            for qt in range(NQC):
                transpose_grp(7, qt)

        if phases == "p0p1":
            return
        pctx.close()  # QA/KTb/WQ/WK/KTe dead -> release SBUF

        # ---- P2: y = OA^T.T @ WO (OAn holds OA^T in-place) ----
        with (
            tc.tile_pool(name="p2", bufs=1) as p2,
            tc.tile_pool(name="ybuf", bufs=2) as ybuf,
            tc.tile_pool(name="psY", bufs=2, space="PSUM") as psY,
        ):
            WO = p2.tile([128, 8, D], F16)
            nc.sync.dma_start(out=WO, in_=wo3)

            for qt in range(NQC):
                yps = psY.tile([128, D], F32, tag="y")
                for j in range(8):
                    for en in range(2):
                        nc.tensor.matmul(
                            yps[:, en * 512 : (en + 1) * 512],
                            OAn[:, qt, j * 128 : (j + 1) * 128],
                            WO[:, j, en * 512 : (en + 1) * 512],
                            start=(j == 0),
                            stop=(j == 7),
                        )
                yb = ybuf.tile([128, D], F32, tag="yb")
                nc.vector.tensor_copy(out=yb, in_=yps)
                nc.sync.dma_start(out=y[qt * 128 : (qt + 1) * 128, :], in_=yb)


def _get_nc(phases="all"):
    if phases in _NC_CACHE:
        return _NC_CACHE[phases]
    nc = bacc.Bacc("TRN2", target_bir_lowering=False)
    t_in = {
        "qT": nc.dram_tensor("qT", [D, Q], F16, kind="ExternalInput"),
        "kT": nc.dram_tensor("kT", [D, S], F16, kind="ExternalInput"),
        "vT": nc.dram_tensor("vT", [D, S], F16, kind="ExternalInput"),
        "mbT": nc.dram_tensor("mbT", [S, Q], F16, kind="ExternalInput"),
        "wqT": nc.dram_tensor("wqT", [D, D], F16, kind="ExternalInput"),
        "wkT": nc.dram_tensor("wkT", [D, D], F16, kind="ExternalInput"),
        "wvT": nc.dram_tensor("wvT", [D, D], F16, kind="ExternalInput"),
        "woT": nc.dram_tensor("woT", [D, D], F16, kind="ExternalInput"),
    }
    t_out = {"y": nc.dram_tensor("y", [Q, D], F32, kind="ExternalOutput")}
    with tile.TileContext(nc) as tc:
        _build_kernel(tc, t_in, t_out, phases=phases)
    nc.compile()
    _NC_CACHE[phases] = nc
    return nc


def _in_maps(inputs):
    q = np.asarray(inputs["query"], np.float32)
    k = np.asarray(inputs["key"], np.float32)
    v = np.asarray(inputs["value"], np.float32)
    mask = np.asarray(inputs["mask"], np.int32)
    f16 = np.float16
    wqT = np.ascontiguousarray(np.asarray(inputs["wq"], np.float32).T).astype(f16)
    wkT = np.ascontiguousarray(np.asarray(inputs["wk"], np.float32).T).astype(f16)
    wvT = np.ascontiguousarray(np.asarray(inputs["wv"], np.float32).T).astype(f16)
    woT = np.ascontiguousarray(np.asarray(inputs["w_out"], np.float32).T).astype(f16)
    maps = []
    for c in range(NCORES):
        b, qh = c // 2, c % 2
        sl = slice(qh * Q, (qh + 1) * Q)
        maps.append(
            {
                "qT": np.ascontiguousarray(q[b].T[:, sl]).astype(f16),
                "kT": np.ascontiguousarray(k[b].T).astype(f16),
                "vT": np.ascontiguousarray(v[b].T).astype(f16),
                "mbT": np.ascontiguousarray(
                    (1 - mask[b].T[:, sl]).astype(f16)
                ),
                "wqT": wqT,
                "wkT": wkT,
                "wvT": wvT,
                "woT": woT,
            }
        )
    return maps


def _gather(res):
    outs = [res.results[c]["y"] for c in range(NCORES)]
    return np.stack(
        [np.concatenate([outs[2 * b], outs[2 * b + 1]], axis=0) for b in range(B)]
    )


def kernel(**inputs) -> np.ndarray:
    nc = _get_nc()
    res = run_bass_kernel_spmd(nc, _in_maps(inputs), core_ids=list(range(NCORES)))
    return _gather(res)


def kernel_traced(**inputs):
    """Like kernel() but with NTFF tracing; returns (output, BassKernelResults)."""
    nc = _get_nc()
    res = run_bass_kernel_spmd(
        nc, _in_maps(inputs), core_ids=list(range(NCORES)), trace=True
    )
    return _gather(res), res


# revision 6
# speedup vs baseline: 1.0237x; 1.0025x over previous
"""MultiHeadAttention Bass/Tile kernel for Trainium2, 8 NeuronCores. V2.

Sharding: (batch, query-half) -> 8 cores, zero collectives.
  core c: batch b = c//2, query rows qh = c%2 (1024 rows each).

All inputs are converted to fp16 and pre-transposed on HOST, so every matmul
operand loads with its contraction dim on partitions and DMA traffic is half
of f32. The mask is sent as (1-mask) in fp16 ("keep" multiplier).

Per-core dataflow:
  P0: stage QA/KTb/MB/weights; Q-proj + K-proj for head-pair 0.
  P1: per head-pair eo (8): per head, per k-strip: S^T[k,q] = K_h^T.T @ Q_h^T
      -> exp(S^T/8) fp16 (ACT) -> * (1-mask^T) (DVE) = pb
      -> PV in [q,d] layout: pv[q, qc, d|den] += pb_chunk.T @ V[strip, h]
         (65-wide moving, full 128 output partitions, ones col -> denominator)
      V-proj is software-pipelined INTO (eo0,h0)'s strip loop; Q/K-proj for
      eo+1 interleaved into the strip stream; PV lags scores by one strip so
      the PE never waits on exp.  Head epilogue: r = 1/den (DVE), normalize
      pv -> OAn fp16 via tensor_scalar with per-partition scalar (fused evac).
  P2: PE-transpose OAn -> OA^T (fp16, via identity), y = OA^T.T @ WO per
      q-tile, DMA y straight from PSUM.
"""

import os
import sys

for _p in ("/opt/trn_rl_repo", "/root/.axon_site/_ro/trn_rl_repo"):
    if os.path.isdir(_p) and _p not in sys.path:
        sys.path.insert(0, _p)

from contextlib import ExitStack

import numpy as np

import concourse.tile as tile
from concourse import bacc, masks, mybir
from concourse.bass_utils import run_bass_kernel_spmd

B, S, D = 4, 2048, 1024
H, HD = 16, 64
Q = S // 2  # per-core query rows
NCORES = 8
NKS = S // 128  # 16 k-strips
NQC = Q // 128  # 8 q-chunks

F32 = mybir.dt.float32
F16 = mybir.dt.float16

_NC_CACHE = {}


def _build_kernel(tc, t_in, t_out, phases="all"):
    nc = tc.nc
    qT, kT, vT, mbT = t_in["qT"], t_in["kT"], t_in["vT"], t_in["mbT"]
    wqT, wkT, wvT, woT = t_in["wqT"], t_in["wkT"], t_in["wvT"], t_in["woT"]
    y = t_out["y"]

    qT3 = qT[:, :].rearrange("(po pi) q -> pi po q", pi=128)  # [128, 8, Q]
    kT3 = kT[:, :].rearrange("(po pi) s -> pi po s", pi=128)
    vT3 = vT[:, :].rearrange("(po pi) s -> pi po s", pi=128)
    mb3 = mbT[:, :].rearrange("(ko ki) q -> ki ko q", ki=128)  # [128, 16, Q]
    wq3 = wqT[:, :].rearrange("(po pi) e -> pi po e", pi=128)
    wk3 = wkT[:, :].rearrange("(po pi) e -> pi po e", pi=128)
    wv3 = wvT[:, :].rearrange("(po pi) e -> pi po e", pi=128)
    wo3 = woT[:, :].rearrange("(po pi) e -> pi po e", pi=128)

    with ExitStack() as ctx:
        # ---- persistent SBUF ----
        p1 = ctx.enter_context(tc.tile_pool(name="persist1", bufs=1))
        QT = p1.tile([128, 8, Q], F16)  # Q^T[e%128, e//128, q]
        V = p1.tile([128, NKS, H, 66], F16)  # [s%128, strip, h, d | one | pad]
        MB = p1.tile([128, NKS, Q], F16)  # (1-mask)^T strips
        OAn = p1.tile([128, NQC, D], F16)  # normalized attn out [q%128, qc, (h d)]
        ident = p1.tile([128, 128], F16)  # for PE transposes

        pctx = ctx.enter_context(ExitStack())
        p2p = pctx.enter_context(tc.tile_pool(name="persist2", bufs=1))
        QA = p2p.tile([128, 8, Q], F16)  # q^T staged
        KTb = p2p.tile([128, 8, S], F16)  # k^T staged

        nc.vector.memset(V[:, :, :, 64:65], 1.0)
        masks.make_identity(nc, ident[:, :])

        kte_pool = pctx.enter_context(tc.tile_pool(name="kte", bufs=2))
        # per-eo slices of wq/wk (only 128 e-cols needed per head-pair)
        wqp = pctx.enter_context(tc.tile_pool(name="wqp", bufs=2))
        wkp = pctx.enter_context(tc.tile_pool(name="wkp", bufs=2))

        WQe, WKe = {}, {}

        def load_w(eo):
            WQe[eo] = wqp.tile([128, 8, 128], F16, tag="wq", name=f"wq{eo}")
            nc.sync.dma_start(
                out=WQe[eo], in_=wq3[:, :, eo * 128 : (eo + 1) * 128]
            )
            WKe[eo] = wkp.tile([128, 8, 128], F16, tag="wk", name=f"wk{eo}")
            nc.sync.dma_start(
                out=WKe[eo], in_=wk3[:, :, eo * 128 : (eo + 1) * 128]
            )

        with (
            tc.tile_pool(name="wv", bufs=1) as wvp,
            tc.tile_pool(name="va", bufs=2) as vap,
            tc.tile_pool(name="pbuf", bufs=11) as pbuf,
            tc.tile_pool(name="rbuf", bufs=2) as rbuf,
            tc.tile_pool(name="psS", bufs=2, space="PSUM") as psS,  # 4 banks
            tc.tile_pool(name="psPV", bufs=1, space="PSUM") as psPV,  # 2
            tc.tile_pool(name="pj", bufs=1, space="PSUM") as pj,  # 2 banks
        ):
            WV = wvp.tile([128, 8, D], F16)

            KTe = {}  # eo -> [128, S] f16 K^T chunk (ring of 2)

            def q_proj_grp(eo):
                # QT[e-chunk eo, :] = sum_d wq[d, e] qa[d, q]
                ps = pj.tile([128, Q], F32, tag="pj", name="qps")
                for qn in range(2):
                    for dc in range(8):
                        nc.tensor.matmul(
                            ps[:, qn * 512 : (qn + 1) * 512],
                            WQe[eo][:, dc, :],
                            QA[:, dc, qn * 512 : (qn + 1) * 512],
                            start=(dc == 0),
                            stop=(dc == 7),
                        )
                nc.vector.tensor_copy(out=QT[:, eo, :], in_=ps)

            def k_proj_grp(eo, snp):
                ps = pj.tile([128, Q], F32, tag="pj", name="kps")
                for sn2 in range(2):
                    sn = snp * 2 + sn2
                    for dc in range(8):
                        nc.tensor.matmul(
                            ps[:, sn2 * 512 : (sn2 + 1) * 512],
                            WKe[eo][:, dc, :],
                            KTb[:, dc, sn * 512 : (sn + 1) * 512],
                            start=(dc == 0),
                            stop=(dc == 7),
                        )
                nc.vector.tensor_copy(
                    out=KTe[eo][:, snp * 1024 : (snp + 1) * 1024], in_=ps
                )

            VA = {}

            def load_va(sn):
                VA[sn] = vap.tile([128, 8, 256], F16, tag="va", name=f"va{sn}")
                nc.sync.dma_start(
                    out=VA[sn], in_=vT3[:, :, sn * 256 : (sn + 1) * 256]
                )

            def v_proj_grp(st):
                # V[strip st, :, :] = sum_d v^T[d, s].T @ wv[d, e]
                ps = pj.tile([128, D], F32, tag="pj", name="vps")
                va, stl = VA[st // 2], st % 2
                for en in range(2):
                    for dc in range(8):
                        nc.tensor.matmul(
                            ps[:, en * 512 : (en + 1) * 512],
                            va[:, dc, stl * 128 : (stl + 1) * 128],
                            WV[:, dc, en * 512 : (en + 1) * 512],
                            start=(dc == 0),
                            stop=(dc == 7),
                        )
                nc.scalar.copy(
                    out=V[:, st, :, 0:64],
                    in_=ps[:, :].rearrange("p (h d) -> p h d", h=16),
                )

            # ---- P0: stage + projections for head-pair 0 ----
            # DMAs ordered by first use so the PE starts ASAP: wk0/wq0 and
            # the first KTb chunk unblock k_proj(0,0) within a few us.
            nc.sync.dma_start(
                out=KTb[:, :, 0:512], in_=kT3[:, :, 0:512]
            )
            load_w(0)
            nc.sync.dma_start(out=QA, in_=qT3)
            for sn in range(1, 4):
                nc.sync.dma_start(
                    out=KTb[:, :, sn * 512 : (sn + 1) * 512],
                    in_=kT3[:, :, sn * 512 : (sn + 1) * 512],
                )
            load_w(1)
            nc.sync.dma_start(out=WV, in_=wv3)
            load_va(0)
            load_va(1)
            nc.sync.dma_start(out=MB[:, 0:4, :], in_=mb3[:, 0:4, :])
            nc.sync.dma_start(out=MB[:, 4:16, :], in_=mb3[:, 4:16, :])
            KTe[0] = kte_pool.tile([128, S], F16, tag="kte", name="kte0")
            for snp in range(2):
                k_proj_grp(0, snp)
            q_proj_grp(0)

            if phases == "p0":
                return

            # ---- P1 ----
            # filler: PE work + prefetch DMAs to interleave into the strip
            # stream.  DMA items lead their consumers by >= one group.
            def filler_gen():
                # eo0 h0: V-projection, strip-by-strip (strip st done well
                # before PV(h0, st) consumes it at slot st+2).
                for sn in range(8):
                    if sn + 2 < 8:
                        yield ("vd", sn + 2)
                    for stl in range(2):
                        yield ("v", sn * 2 + stl)
                # eo 1..7 prep; weight slices for eo+1 prefetched while eo's
                # groups are computed.
                for eo in range(1, 8):
                    if eo + 1 < 8:
                        yield ("w", eo + 1)
                    KTe[eo] = kte_pool.tile(
                        [128, S], F16, tag="kte", name=f"kte{eo}"
                    )
                    for snp in range(2):
                        yield ("k", eo, snp)
                    yield ("q", eo)

            fill = filler_gen()

            def t_head(hi):
                # estimated start time (ns) of head hi in the floor schedule
                return 13000 + (58000 if hi >= 1 else 0) + max(0, hi - 1) * 18000

            def do_fill(n):
                for _ in range(n):
                    item = next(fill, None)
                    if item is None:
                        return
                    kind = item[0]
                    if kind == "v":
                        v_proj_grp(item[1])
                    elif kind == "vd":
                        load_va(item[1])
                    elif kind == "w":
                        eo2 = item[1]
                        with tc.tile_wait_until(t_head(max(0, 2 * eo2 - 4)) / 1e6):
                            load_w(eo2)
                    elif kind == "k":
                        eo2, snp = item[1], item[2]
                        base = t_head(2 * eo2 - 2 if eo2 > 1 else 1)
                        with tc.tile_wait_until((base + snp * 5000) / 1e6):
                            k_proj_grp(eo2, snp)
                    else:
                        eo2 = item[1]
                        base = t_head(2 * eo2 - 2 if eo2 > 1 else 1)
                        with tc.tile_wait_until((base + 11000) / 1e6):
                            q_proj_grp(eo2)

            def transpose_grp(j, qt):
                # in-place: OAn chunk (j, qt) -> its own transpose via PSUM
                tp = pj.tile([128, 128], F16, tag="pj", name="tp")
                nc.tensor.transpose(
                    tp, OAn[:, qt, j * 128 : (j + 1) * 128], ident[:, :]
                )
                nc.vector.tensor_copy(
                    out=OAn[:, qt, j * 128 : (j + 1) * 128], in_=tp
                )

            def do_pv(pv, h, ks, pb):
                # pv is 2 psum banks; 4 qc-slabs share a 2KB zero region ->
                # start only on the first matmul touching the bank (zeroes
                # the whole region), stop on the last.
                for qc in range(NQC):
                    nc.tensor.matmul(
                        pv[:, qc, 0:65],
                        pb[:, qc * 128 : (qc + 1) * 128],
                        V[:, ks, h, 0:65],
                        start=(ks == 0 and qc % 4 == 0),
                        stop=(ks == NKS - 1 and qc % 4 == 3),
                    )

            def epilogue(pv, h):
                # normalize pv -> OAn, fused with the PSUM evacuation.  On
                # ACT (Copy with per-partition scale) so the DVE queue (busy
                # with masks) doesn't delay the next head's PV.
                rsb = rbuf.tile([128, NQC], F32, tag="r")
                nc.vector.reciprocal(
                    out=rsb,
                    in_=pv[:, :, 64:65].rearrange("p a b -> p (a b)"),
                )
                for qc in range(NQC):
                    nc.vector.tensor_scalar(
                        out=OAn[:, qc, h * 64 : (h + 1) * 64],
                        in0=pv[:, qc, 0:64],
                        scalar1=rsb[:, qc : qc + 1],
                        scalar2=None,
                        op0=mybir.AluOpType.mult,
                    )

            # Flat software pipeline over all (head, strip) slots.  The PV
            # queue (lag 2) spans head boundaries so the next head's scores
            # never sit behind the previous head's tail PVs in the in-order
            # PE queue.  Epilogue runs when a head's last PV retires; the pv
            # psum buffer is re-allocated when the next head's first PV pops.
            state = {"pv": None, "epi": None}

            def pop_pv(pend):
                h2, ks2, pb2 = pend.pop(0)
                if ks2 == 0:
                    if state["epi"] is not None:
                        epilogue(*state["epi"])
                        state["epi"] = None
                    state["pv"] = psPV.tile(
                        [128, NQC, 128], F32, tag="pv", name="pv"
                    )
                do_pv(state["pv"], h2, ks2, pb2)
                if ks2 == NKS - 1:
                    state["epi"] = (state["pv"], h2)

            pend = []
            for slot, (eo, hl, ks) in enumerate(
                (e, l, k) for e in range(8) for l in range(2) for k in range(NKS)
            ):
                h = 2 * eo + hl
                hp = 64 * hl
                # PE-ready work first: while ACT finishes exp(slot-2)
                # (freeing the sps buffer this slot's scores need), the
                # in-order PE queue chews through filler + lagged PV.
                if eo == 0 and hl == 0:
                    do_fill(2)
                elif (ks % 2) == 1:
                    do_fill(1)
                if len(pend) >= 7:
                    pop_pv(pend)
                # in-place transposes of head-pair eo-1 (its OAn chunk is
                # final: head 2eo-1's epilogue was emitted at slot 2 above)
                if hl == 0 and eo >= 1 and 8 <= ks < 8 + NQC:
                    transpose_grp(eo - 1, ks - 8)
                sps = psS.tile([128, Q], F32, tag="ps", name="sps")
                lhsT = KTe[eo][hp : hp + 64, ks * 128 : (ks + 1) * 128]
                for qn in range(2):
                    nc.tensor.matmul(
                        sps[:, qn * 512 : (qn + 1) * 512],
                        lhsT,
                        QT[hp : hp + 64, eo, qn * 512 : (qn + 1) * 512],
                        start=True,
                        stop=True,
                    )
                pb = pbuf.tile([128, Q], F16, tag="pb")
                nc.scalar.activation(
                    out=pb,
                    in_=sps,
                    func=mybir.ActivationFunctionType.Exp,
                    scale=0.125,
                )
                nc.vector.tensor_tensor(
                    out=pb, in0=pb, in1=MB[:, ks, :],
                    op=mybir.AluOpType.mult,
                )
                pend.append((h, ks, pb))
            while pend:
                pop_pv(pend)
            do_fill(999)  # drain any leftover filler (incl. transposes)
            epilogue(*state["epi"])
            # last head-pair's transposes (OAn chunk 7 final after epilogue)
            for qt in range(NQC):
                transpose_grp(7, qt)

        if phases == "p0p1":
            return
        pctx.close()  # QA/KTb/WQ/WK/KTe dead -> release SBUF

        # ---- P2: y = OA^T.T @ WO (OAn holds OA^T in-place) ----
        with (
            tc.tile_pool(name="p2", bufs=1) as p2,
            tc.tile_pool(name="ybuf", bufs=2) as ybuf,
            tc.tile_pool(name="psY", bufs=2, space="PSUM") as psY,
        ):
            WO = p2.tile([128, 8, D], F16)
            nc.sync.dma_start(out=WO, in_=wo3)

            for qt in range(NQC):
                yps = psY.tile([128, D], F32, tag="y")
                for j in range(8):
                    for en in range(2):
                        nc.tensor.matmul(
                            yps[:, en * 512 : (en + 1) * 512],
                            OAn[:, qt, j * 128 : (j + 1) * 128],
                            WO[:, j, en * 512 : (en + 1) * 512],
                            start=(j == 0),
                            stop=(j == 7),
                        )
                yb = ybuf.tile([128, D], F32, tag="yb")
                nc.vector.tensor_copy(out=yb, in_=yps)
                nc.sync.dma_start(out=y[qt * 128 : (qt + 1) * 128, :], in_=yb)


def _get_nc(phases="all"):
    if phases in _NC_CACHE:
        return _NC_CACHE[phases]
    nc = bacc.Bacc("TRN2", target_bir_lowering=False)
    t_in = {
        "qT": nc.dram_tensor("qT", [D, Q], F16, kind="ExternalInput"),
        "kT": nc.dram_tensor("kT", [D, S], F16, kind="ExternalInput"),
        "vT": nc.dram_tensor("vT", [D, S], F16, kind="ExternalInput"),
        "mbT": nc.dram_tensor("mbT", [S, Q], F16, kind="ExternalInput"),
        "wqT": nc.dram_tensor("wqT", [D, D], F16, kind="ExternalInput"),
        "wkT": nc.dram_tensor("wkT", [D, D], F16, kind="ExternalInput"),
        "wvT": nc.dram_tensor("wvT", [D, D], F16, kind="ExternalInput"),
        "woT": nc.dram_tensor("woT", [D, D], F16, kind="ExternalInput"),
    }
    t_out = {"y": nc.dram_tensor("y", [Q, D], F32, kind="ExternalOutput")}
    with tile.TileContext(nc) as tc:
        _build_kernel(tc, t_in, t_out, phases=phases)
    nc.compile()
    _NC_CACHE[phases] = nc
    return nc


def _in_maps(inputs):
    q = np.asarray(inputs["query"], np.float32)
    k = np.asarray(inputs["key"], np.float32)
    v = np.asarray(inputs["value"], np.float32)
    mask = np.asarray(inputs["mask"], np.int32)
    f16 = np.float16
    wqT = np.ascontiguousarray(np.asarray(inputs["wq"], np.float32).T).astype(f16)
    wkT = np.ascontiguousarray(np.asarray(inputs["wk"], np.float32).T).astype(f16)
    wvT = np.ascontiguousarray(np.asarray(inputs["wv"], np.float32).T).astype(f16)
    woT = np.ascontiguousarray(np.asarray(inputs["w_out"], np.float32).T).astype(f16)
    maps = []
    for c in range(NCORES):
        b, qh = c // 2, c % 2
        sl = slice(qh * Q, (qh + 1) * Q)
        maps.append(
            {
                "qT": np.ascontiguousarray(q[b].T[:, sl]).astype(f16),
                "kT": np.ascontiguousarray(k[b].T).astype(f16),
                "vT": np.ascontiguousarray(v[b].T).astype(f16),
                "mbT": np.ascontiguousarray(
                    (1 - mask[b].T[:, sl]).astype(f16)
                ),
                "wqT": wqT,
                "wkT": wkT,
                "wvT": wvT,
                "woT": woT,
            }
        )
    return maps


def _gather(res):
    outs = [res.results[c]["y"] for c in range(NCORES)]
    return np.stack(
        [np.concatenate([outs[2 * b], outs[2 * b + 1]], axis=0) for b in range(B)]
    )


def kernel(**inputs) -> np.ndarray:
    nc = _get_nc()
    res = run_bass_kernel_spmd(nc, _in_maps(inputs), core_ids=list(range(NCORES)))
    return _gather(res)


def kernel_traced(**inputs):
    """Like kernel() but with NTFF tracing; returns (output, BassKernelResults)."""
    nc = _get_nc()
    res = run_bass_kernel_spmd(
        nc, _in_maps(inputs), core_ids=list(range(NCORES)), trace=True
    )
    return _gather(res), res


# revision 7
# speedup vs baseline: 1.0267x; 1.0030x over previous
"""MultiHeadAttention Bass/Tile kernel for Trainium2, 8 NeuronCores. V2.

Sharding: (batch, query-half) -> 8 cores, zero collectives.
  core c: batch b = c//2, query rows qh = c%2 (1024 rows each).

All inputs are converted to fp16 and pre-transposed on HOST, so every matmul
operand loads with its contraction dim on partitions and DMA traffic is half
of f32. The mask is sent as (1-mask) in fp16 ("keep" multiplier).

Per-core dataflow:
  P0: stage QA/KTb/MB/weights; Q-proj + K-proj for head-pair 0.
  P1: per head-pair eo (8): per head, per k-strip: S^T[k,q] = K_h^T.T @ Q_h^T
      -> exp(S^T/8) fp16 (ACT) -> * (1-mask^T) (DVE) = pb
      -> PV in [q,d] layout: pv[q, qc, d|den] += pb_chunk.T @ V[strip, h]
         (65-wide moving, full 128 output partitions, ones col -> denominator)
      V-proj is software-pipelined INTO (eo0,h0)'s strip loop; Q/K-proj for
      eo+1 interleaved into the strip stream; PV lags scores by one strip so
      the PE never waits on exp.  Head epilogue: r = 1/den (DVE), normalize
      pv -> OAn fp16 via tensor_scalar with per-partition scalar (fused evac).
  P2: PE-transpose OAn -> OA^T (fp16, via identity), y = OA^T.T @ WO per
      q-tile, DMA y straight from PSUM.
"""

import os
import sys

for _p in ("/opt/trn_rl_repo", "/root/.axon_site/_ro/trn_rl_repo"):
    if os.path.isdir(_p) and _p not in sys.path:
        sys.path.insert(0, _p)

from contextlib import ExitStack

import numpy as np

import concourse.tile as tile
from concourse import bacc, masks, mybir
from concourse.bass_utils import run_bass_kernel_spmd

B, S, D = 4, 2048, 1024
H, HD = 16, 64
Q = S // 2  # per-core query rows
NCORES = 8
NKS = S // 128  # 16 k-strips
NQC = Q // 128  # 8 q-chunks

F32 = mybir.dt.float32
F16 = mybir.dt.float16

_NC_CACHE = {}


def _build_kernel(tc, t_in, t_out, phases="all"):
    nc = tc.nc
    qT, kT, vT, mbT = t_in["qT"], t_in["kT"], t_in["vT"], t_in["mbT"]
    wqT, wkT, wvT, woT = t_in["wqT"], t_in["wkT"], t_in["wvT"], t_in["woT"]
    y = t_out["y"]

    qT3 = qT[:, :].rearrange("(po pi) q -> pi po q", pi=128)  # [128, 8, Q]
    kT3 = kT[:, :].rearrange("(po pi) s -> pi po s", pi=128)
    vT3 = vT[:, :].rearrange("(po pi) s -> pi po s", pi=128)
    mb3 = mbT[:, :].rearrange("(ko ki) q -> ki ko q", ki=128)  # [128, 16, Q]
    wq3 = wqT[:, :].rearrange("(po pi) e -> pi po e", pi=128)
    wk3 = wkT[:, :].rearrange("(po pi) e -> pi po e", pi=128)
    wv3 = wvT[:, :].rearrange("(po pi) e -> pi po e", pi=128)
    wo3 = woT[:, :].rearrange("(po pi) e -> pi po e", pi=128)

    with ExitStack() as ctx:
        # ---- persistent SBUF ----
        p1 = ctx.enter_context(tc.tile_pool(name="persist1", bufs=1))

        V = p1.tile([128, NKS, H, 66], F16)  # [s%128, strip, h, d | one | pad]
        MB = p1.tile([128, NKS, Q], F16)  # (1-mask)^T strips
        OAn = p1.tile([128, NQC, D], F16)  # normalized attn out [q%128, qc, (h d)]
        ident = p1.tile([128, 128], F16)  # for PE transposes

        pctx = ctx.enter_context(ExitStack())
        p2p = pctx.enter_context(tc.tile_pool(name="persist2", bufs=1))
        QA = p2p.tile([128, 8, Q], F16)  # q^T staged
        KTb = p2p.tile([128, 8, S], F16)  # k^T staged

        nc.vector.memset(V[:, :, :, 64:65], 1.0)
        masks.make_identity(nc, ident[:, :])

        kte_pool = pctx.enter_context(tc.tile_pool(name="kte", bufs=2))
        qte_pool = pctx.enter_context(tc.tile_pool(name="qte", bufs=3))
        # per-eo slices of wq/wk (only 128 e-cols needed per head-pair)
        wqp = pctx.enter_context(tc.tile_pool(name="wqp", bufs=2))
        wkp = pctx.enter_context(tc.tile_pool(name="wkp", bufs=2))

        WQe, WKe = {}, {}

        def load_w(eo):
            WQe[eo] = wqp.tile([128, 8, 128], F16, tag="wq", name=f"wq{eo}")
            nc.sync.dma_start(
                out=WQe[eo], in_=wq3[:, :, eo * 128 : (eo + 1) * 128]
            )
            WKe[eo] = wkp.tile([128, 8, 128], F16, tag="wk", name=f"wk{eo}")
            nc.sync.dma_start(
                out=WKe[eo], in_=wk3[:, :, eo * 128 : (eo + 1) * 128]
            )

        with (
            tc.tile_pool(name="wv", bufs=1) as wvp,
            tc.tile_pool(name="va", bufs=3) as vap,
            tc.tile_pool(name="pbuf", bufs=12) as pbuf,
            tc.tile_pool(name="rbuf", bufs=2) as rbuf,
            tc.tile_pool(name="psS", bufs=2, space="PSUM") as psS,  # 4 banks
            tc.tile_pool(name="psPV", bufs=1, space="PSUM") as psPV,  # 2
            tc.tile_pool(name="pj", bufs=1, space="PSUM") as pj,  # 2 banks
        ):
            WV = wvp.tile([128, 8, D], F16)

            KTe = {}  # eo -> [128, S] f16 K^T chunk (ring of 2)
            QTe = {}  # eo -> [128, Q] f16 Q^T chunk (ring of 3)

            def q_proj_grp(eo):
                # QTe[eo][e, :] = sum_d wq[d, e] qa[d, q]
                QTe[eo] = qte_pool.tile([128, Q], F16, tag="qte", name=f"qte{eo}")
                ps = pj.tile([128, Q], F32, tag="pj", name="qps")
                for qn in range(2):
                    for dc in range(8):
                        nc.tensor.matmul(
                            ps[:, qn * 512 : (qn + 1) * 512],
                            WQe[eo][:, dc, :],
                            QA[:, dc, qn * 512 : (qn + 1) * 512],
                            start=(dc == 0),
                            stop=(dc == 7),
                        )
                nc.vector.tensor_copy(out=QTe[eo], in_=ps)

            def k_proj_grp(eo, snp):
                ps = pj.tile([128, Q], F32, tag="pj", name="kps")
                for sn2 in range(2):
                    sn = snp * 2 + sn2
                    for dc in range(8):
                        nc.tensor.matmul(
                            ps[:, sn2 * 512 : (sn2 + 1) * 512],
                            WKe[eo][:, dc, :],
                            KTb[:, dc, sn * 512 : (sn + 1) * 512],
                            start=(dc == 0),
                            stop=(dc == 7),
                        )
                nc.vector.tensor_copy(
                    out=KTe[eo][:, snp * 1024 : (snp + 1) * 1024], in_=ps
                )

            VA = {}

            def load_va(sn):
                VA[sn] = vap.tile([128, 8, 256], F16, tag="va", name=f"va{sn}")
                nc.sync.dma_start(
                    out=VA[sn], in_=vT3[:, :, sn * 256 : (sn + 1) * 256]
                )

            def v_proj_grp(st):
                # V[strip st, :, :] = sum_d v^T[d, s].T @ wv[d, e]
                ps = pj.tile([128, D], F32, tag="pj", name="vps")
                va, stl = VA[st // 2], st % 2
                for en in range(2):
                    for dc in range(8):
                        nc.tensor.matmul(
                            ps[:, en * 512 : (en + 1) * 512],
                            va[:, dc, stl * 128 : (stl + 1) * 128],
                            WV[:, dc, en * 512 : (en + 1) * 512],
                            start=(dc == 0),
                            stop=(dc == 7),
                        )
                nc.scalar.copy(
                    out=V[:, st, :, 0:64],
                    in_=ps[:, :].rearrange("p (h d) -> p h d", h=16),
                )

            # ---- P0: stage + projections for head-pair 0 ----
            # DMAs ordered by first use so the PE starts ASAP: wk0/wq0 and
            # the first KTb chunk unblock k_proj(0,0) within a few us.
            nc.sync.dma_start(
                out=KTb[:, :, 0:512], in_=kT3[:, :, 0:512]
            )
            load_w(0)
            nc.sync.dma_start(out=QA, in_=qT3)
            for sn in range(1, 4):
                nc.sync.dma_start(
                    out=KTb[:, :, sn * 512 : (sn + 1) * 512],
                    in_=kT3[:, :, sn * 512 : (sn + 1) * 512],
                )
            load_w(1)
            nc.sync.dma_start(out=WV, in_=wv3)
            load_va(0)
            load_va(1)
            nc.sync.dma_start(out=MB[:, 0:4, :], in_=mb3[:, 0:4, :])
            nc.sync.dma_start(out=MB[:, 4:16, :], in_=mb3[:, 4:16, :])
            KTe[0] = kte_pool.tile([128, S], F16, tag="kte", name="kte0")
            for snp in range(2):
                k_proj_grp(0, snp)
            q_proj_grp(0)

            if phases == "p0":
                return

            # ---- P1 ----
            # filler: PE work + prefetch DMAs to interleave into the strip
            # stream.  DMA items lead their consumers by >= one group.
            def filler_gen():
                # eo0 h0: V-projection, strip-by-strip (strip st done well
                # before PV(h0, st) consumes it at slot st+2).
                for sn in range(8):
                    if sn + 2 < 8:
                        yield ("vd", sn + 2)
                    for stl in range(2):
                        yield ("v", sn * 2 + stl)
                # eo 1..7 prep; weight slices for eo+1 prefetched while eo's
                # groups are computed.
                for eo in range(1, 8):
                    if eo + 1 < 8:
                        yield ("w", eo + 1)
                    KTe[eo] = kte_pool.tile(
                        [128, S], F16, tag="kte", name=f"kte{eo}"
                    )
                    for snp in range(2):
                        yield ("k", eo, snp)
                    yield ("q", eo)

            fill = filler_gen()

            def t_head(hi):
                # estimated start time (ns) of head hi in the floor schedule
                return 13000 + (58000 if hi >= 1 else 0) + max(0, hi - 1) * 18000

            def do_fill(n):
                for _ in range(n):
                    item = next(fill, None)
                    if item is None:
                        return
                    kind = item[0]
                    if kind == "v":
                        v_proj_grp(item[1])
                    elif kind == "vd":
                        load_va(item[1])
                    elif kind == "w":
                        eo2 = item[1]
                        with tc.tile_wait_until(t_head(max(0, 2 * eo2 - 4)) / 1e6):
                            load_w(eo2)
                    elif kind == "k":
                        eo2, snp = item[1], item[2]
                        base = t_head(2 * eo2 - 2 if eo2 > 1 else 1)
                        with tc.tile_wait_until((base + snp * 5000) / 1e6):
                            k_proj_grp(eo2, snp)
                    else:
                        eo2 = item[1]
                        base = t_head(2 * eo2 - 2 if eo2 > 1 else 1)
                        with tc.tile_wait_until((base + 11000) / 1e6):
                            q_proj_grp(eo2)

            def transpose_grp(j, qt):
                # in-place: OAn chunk (j, qt) -> its own transpose via PSUM
                tp = pj.tile([128, 128], F16, tag="pj", name="tp")
                nc.tensor.transpose(
                    tp, OAn[:, qt, j * 128 : (j + 1) * 128], ident[:, :]
                )
                nc.vector.tensor_copy(
                    out=OAn[:, qt, j * 128 : (j + 1) * 128], in_=tp
                )

            def do_pv(pv, h, ks, pb):
                # pv is 2 psum banks; 4 qc-slabs share a 2KB zero region ->
                # start only on the first matmul touching the bank (zeroes
                # the whole region), stop on the last.
                for qc in range(NQC):
                    nc.tensor.matmul(
                        pv[:, qc, 0:65],
                        pb[:, qc * 128 : (qc + 1) * 128],
                        V[:, ks, h, 0:65],
                        start=(ks == 0 and qc % 4 == 0),
                        stop=(ks == NKS - 1 and qc % 4 == 3),
                    )

            def epilogue(pv, h):
                # normalize pv -> OAn, fused with the PSUM evacuation.  On
                # ACT (Copy with per-partition scale) so the DVE queue (busy
                # with masks) doesn't delay the next head's PV.
                rsb = rbuf.tile([128, NQC], F32, tag="r")
                nc.vector.reciprocal(
                    out=rsb,
                    in_=pv[:, :, 64:65].rearrange("p a b -> p (a b)"),
                )
                for qc in range(NQC):
                    nc.vector.tensor_scalar(
                        out=OAn[:, qc, h * 64 : (h + 1) * 64],
                        in0=pv[:, qc, 0:64],
                        scalar1=rsb[:, qc : qc + 1],
                        scalar2=None,
                        op0=mybir.AluOpType.mult,
                    )

            # Flat software pipeline over all (head, strip) slots.  The PV
            # queue (lag 2) spans head boundaries so the next head's scores
            # never sit behind the previous head's tail PVs in the in-order
            # PE queue.  Epilogue runs when a head's last PV retires; the pv
            # psum buffer is re-allocated when the next head's first PV pops.
            state = {"pv": None, "epi": None}

            def pop_pv(pend):
                h2, ks2, pb2 = pend.pop(0)
                if ks2 == 0:
                    if state["epi"] is not None:
                        epilogue(*state["epi"])
                        state["epi"] = None
                    state["pv"] = psPV.tile(
                        [128, NQC, 128], F32, tag="pv", name="pv"
                    )
                do_pv(state["pv"], h2, ks2, pb2)
                if ks2 == NKS - 1:
                    state["epi"] = (state["pv"], h2)

            pend = []
            for slot, (eo, hl, ks) in enumerate(
                (e, l, k) for e in range(8) for l in range(2) for k in range(NKS)
            ):
                h = 2 * eo + hl
                hp = 64 * hl
                # PE-ready work first: while ACT finishes exp(slot-2)
                # (freeing the sps buffer this slot's scores need), the
                # in-order PE queue chews through filler + lagged PV.
                if eo == 0 and hl == 0:
                    do_fill(2)
                elif (ks % 2) == 1:
                    do_fill(1)
                if len(pend) >= 8:
                    pop_pv(pend)
                # in-place transposes of head-pair eo-1 (its OAn chunk is
                # final: head 2eo-1's epilogue was emitted at slot 2 above)
                if hl == 0 and eo >= 1 and 8 <= ks < 8 + NQC:
                    transpose_grp(eo - 1, ks - 8)
                sps = psS.tile([128, Q], F32, tag="ps", name="sps")
                lhsT = KTe[eo][hp : hp + 64, ks * 128 : (ks + 1) * 128]
                for qn in range(2):
                    nc.tensor.matmul(
                        sps[:, qn * 512 : (qn + 1) * 512],
                        lhsT,
                        QTe[eo][hp : hp + 64, qn * 512 : (qn + 1) * 512],
                        start=True,
                        stop=True,
                    )
                pb = pbuf.tile([128, Q], F16, tag="pb")
                nc.scalar.activation(
                    out=pb,
                    in_=sps,
                    func=mybir.ActivationFunctionType.Exp,
                    scale=0.125,
                )
                nc.vector.tensor_tensor(
                    out=pb, in0=pb, in1=MB[:, ks, :],
                    op=mybir.AluOpType.mult,
                )
                pend.append((h, ks, pb))
            while pend:
                pop_pv(pend)
            do_fill(999)  # drain any leftover filler (incl. transposes)
            epilogue(*state["epi"])
            # last head-pair's transposes (OAn chunk 7 final after epilogue)
            for qt in range(NQC):
                transpose_grp(7, qt)

        if phases == "p0p1":
            return
        pctx.close()  # QA/KTb/WQ/WK/KTe dead -> release SBUF

        # ---- P2: y = OA^T.T @ WO (OAn holds OA^T in-place) ----
        with (
            tc.tile_pool(name="p2", bufs=1) as p2,
            tc.tile_pool(name="ybuf", bufs=2) as ybuf,
            tc.tile_pool(name="psY", bufs=2, space="PSUM") as psY,
        ):
            WO = p2.tile([128, 8, D], F16)
            nc.sync.dma_start(out=WO, in_=wo3)

            for qt in range(NQC):
                yps = psY.tile([128, D], F32, tag="y")
                for j in range(8):
                    for en in range(2):
                        nc.tensor.matmul(
                            yps[:, en * 512 : (en + 1) * 512],
                            OAn[:, qt, j * 128 : (j + 1) * 128],
                            WO[:, j, en * 512 : (en + 1) * 512],
                            start=(j == 0),
                            stop=(j == 7),
                        )
                yb = ybuf.tile([128, D], F32, tag="yb")
                nc.vector.tensor_copy(out=yb, in_=yps)
                nc.sync.dma_start(out=y[qt * 128 : (qt + 1) * 128, :], in_=yb)


def _get_nc(phases="all"):
    if phases in _NC_CACHE:
        return _NC_CACHE[phases]
    nc = bacc.Bacc("TRN2", target_bir_lowering=False)
    t_in = {
        "qT": nc.dram_tensor("qT", [D, Q], F16, kind="ExternalInput"),
        "kT": nc.dram_tensor("kT", [D, S], F16, kind="ExternalInput"),
        "vT": nc.dram_tensor("vT", [D, S], F16, kind="ExternalInput"),
        "mbT": nc.dram_tensor("mbT", [S, Q], F16, kind="ExternalInput"),
        "wqT": nc.dram_tensor("wqT", [D, D], F16, kind="ExternalInput"),
        "wkT": nc.dram_tensor("wkT", [D, D], F16, kind="ExternalInput"),
        "wvT": nc.dram_tensor("wvT", [D, D], F16, kind="ExternalInput"),
        "woT": nc.dram_tensor("woT", [D, D], F16, kind="ExternalInput"),
    }
    t_out = {"y": nc.dram_tensor("y", [Q, D], F32, kind="ExternalOutput")}
    with tile.TileContext(nc) as tc:
        _build_kernel(tc, t_in, t_out, phases=phases)
    nc.compile()
    _NC_CACHE[phases] = nc
    return nc


def _in_maps(inputs):
    q = np.asarray(inputs["query"], np.float32)
    k = np.asarray(inputs["key"], np.float32)
    v = np.asarray(inputs["value"], np.float32)
    mask = np.asarray(inputs["mask"], np.int32)
    f16 = np.float16
    wqT = np.ascontiguousarray(np.asarray(inputs["wq"], np.float32).T).astype(f16)
    wkT = np.ascontiguousarray(np.asarray(inputs["wk"], np.float32).T).astype(f16)
    wvT = np.ascontiguousarray(np.asarray(inputs["wv"], np.float32).T).astype(f16)
    woT = np.ascontiguousarray(np.asarray(inputs["w_out"], np.float32).T).astype(f16)
    maps = []
    for c in range(NCORES):
        b, qh = c // 2, c % 2
        sl = slice(qh * Q, (qh + 1) * Q)
        maps.append(
            {
                "qT": np.ascontiguousarray(q[b].T[:, sl]).astype(f16),
                "kT": np.ascontiguousarray(k[b].T).astype(f16),
                "vT": np.ascontiguousarray(v[b].T).astype(f16),
                "mbT": np.ascontiguousarray(
                    (1 - mask[b].T[:, sl]).astype(f16)
                ),
                "wqT": wqT,
                "wkT": wkT,
                "wvT": wvT,
                "woT": woT,
            }
        )
    return maps


def _gather(res):
    outs = [res.results[c]["y"] for c in range(NCORES)]
    return np.stack(
        [np.concatenate([outs[2 * b], outs[2 * b + 1]], axis=0) for b in range(B)]
    )


def kernel(**inputs) -> np.ndarray:
    nc = _get_nc()
    res = run_bass_kernel_spmd(nc, _in_maps(inputs), core_ids=list(range(NCORES)))
    return _gather(res)


def kernel_traced(**inputs):
    """Like kernel() but with NTFF tracing; returns (output, BassKernelResults)."""
    nc = _get_nc()
    res = run_bass_kernel_spmd(
        nc, _in_maps(inputs), core_ids=list(range(NCORES)), trace=True
    )
    return _gather(res), res
